# revision 1
# baseline (speedup 1.0000x reference)
"""Chamfer-like distance loss on Trainium2 (Bass/Tile), 8-core SPMD.

Problem: depth_pred (4,1,64,64), boundary_gt (4,1,64,64).
  g = sqrt(sobel_x(depth)^2 + sobel_y(depth)^2 + 1e-8)  flattened to (B, N=4096)
  b = boundary flattened (B, 4096)
  d[i,j] = |g_i - b_j|;  out = mean_i min_j d  +  mean_j min_i d

Sharding: core k handles batch k//2, image-row half k%2 (32 rows = 2048 i's).
Each core computes, for its 2048 gradient points vs all 4096 boundary points:
  - rowmin: min_j |g_i - b_j| for each of its i  -> summed into rowsum (128,1)
  - colmin partial: min over its i of |g_i - b_j| for every j -> colmin (128,32)
Host combines: dist1 = sum of all rowsums / 16384; per batch the two cores'
colmin partials are elementwise-min'd, then dist2 = sum / 16384.

On-device design (both chamfer directions are pure free-axis reduces; no
partition-axis reduction anywhere):
  - Sobel runs in transposed layout (image cols on partitions, rows on the
    free axis) from three host-supplied column-shifted slabs, so every op is
    partition-aligned; vertical taps are free-axis shifts.
  - Pass A (rowmin): i-tile t of 16 -> 128 gradient points on partitions,
    boundary points on the free axis. Pass B (colmin): j-tile u of 32 ->
    128 boundary points on partitions (j = p*32+u, so the b_s scalar load
    is one clean strided DMA); colmin lands directly as (128, 32).
  - Every tile is ONE custom DVE instruction, ABS2_MIN_RED_ANT:
        out = min(|in0 - s0|, |in1 - s0|)   (elementwise)
        accum_out = min(s1, min_k out)      (free-axis reduce)
    with the reduced axis split in half across in0/in1 so BOTH DVE read
    ports stream data - 2 fp32 elements/cycle, the port ceiling. s0 is the
    per-partition scalar (g for pass A, b for pass B), s1 seeds/chains the
    accumulator.
  - Broadcast setup: b lower half = rank-1 PE matmul (ones @ b) resident in
    PSUM, b upper half = 8 stride-0 DMAs DRAM->SBUF starting at t~0 (the
    custom op reads one half per port; src0/src1 cannot both be PSUM);
    g = DRAM bounce + 8 stride-0 DMAs. The first two pass-A tiles are
    quarter-chunked and chained through s1 so DVE starts ~10us in.
    DVE ends up ~91% busy; ACT/GPSIMD idle; measured ~104us on HW.
"""
import os
import sys

import numpy as np

for _p in ("/opt/trn_rl_repo", os.path.expanduser("~/.axon_site/_ro/trn_rl_repo")):
    if os.path.isdir(_p) and _p not in sys.path:
        sys.path.insert(0, _p)

import concourse.bass as bass
import concourse.bacc as bacc
import concourse.tile as tile
from concourse import mybir
from concourse.bass_utils import run_bass_kernel_spmd
from concourse import dve_ops
from concourse.dve_spec import Spec, Src0, Src1, C0, C1, maxx, minn, lower, _has_src1
from concourse.dve_uop import DveOpSpec


def _register_absdiff_min_op():
    """Custom DVE op: out = |in0 - s0|, accum_out = min(s1, min_k out).
    Fuses the abs-diff production and the free-axis min reduce into one
    1 elem/cycle DVE instruction."""
    name = "ABS_SUB_MIN_RED_ANT"
    for o in dve_ops.OPS:
        if o.name == name:
            return o

    def _ref(in0, in1, s0, s1, imm2):
        b = np.abs(in0.astype(np.float32) - s0).astype(np.float32)
        acc = np.minimum(
            np.float32(s1) if np.isscalar(s1) else s1.astype(np.float32),
            b.reshape(b.shape[0], -1).min(axis=-1, keepdims=True),
        )
        return b, acc

    spec = Spec(
        body=maxx(Src0 - C0, C0 - Src0),
        accum=minn,
        accum_init=C1,
        reference=_ref,
    )
    op = dve_ops.DveOp(name, spec, subdim=False, uops_sha={})
    row = dve_ops._CUSTOM_DVE_ROW_BASE + len(dve_ops.OPS)
    assert row < 0x20
    dve_ops.OPS.append(op)
    dve_ops.CUSTOM_DVE_SPECS[name] = spec
    dve_ops._SUB_OPCODE_FOR_NAME[name] = row
    for ver in ("v3", "v4"):
        compiled = DveOpSpec(
            name=name, opcode=row, uops=lower(spec, ver=ver),
            rd1_en=_has_src1(spec),
        )
        op.uops_sha[ver] = compiled.sha(ver)
    return op


ABSDIFF_MIN = _register_absdiff_min_op()


def _register_absdiff2_min_op():
    """Two-stream variant: out = min(|in0-s0|, |in1-s0|) elementwise,
    accum_out = min(s1, min_k out). Both DVE read ports stream data, so it
    consumes TWO tensor elements per cycle - the j axis is split in half
    across in0/in1 and the pairwise min happens in-body."""
    name = "ABS2_MIN_RED_ANT"
    for o in dve_ops.OPS:
        if o.name == name:
            return o

    def _ref(in0, in1, s0, s1, imm2):
        b = np.minimum(
            np.abs(in0.astype(np.float32) - s0),
            np.abs(in1.astype(np.float32) - s0),
        ).astype(np.float32)
        acc = np.minimum(
            np.float32(s1) if np.isscalar(s1) else s1.astype(np.float32),
            b.reshape(b.shape[0], -1).min(axis=-1, keepdims=True),
        )
        return b, acc

    spec = Spec(
        body=minn(maxx(Src0 - C0, C0 - Src0), maxx(Src1 - C0, C0 - Src1)),
        accum=minn,
        accum_init=C1,
        reference=_ref,
    )
    op = dve_ops.DveOp(name, spec, subdim=False, uops_sha={})
    row = dve_ops._CUSTOM_DVE_ROW_BASE + len(dve_ops.OPS)
    assert row < 0x20
    dve_ops.OPS.append(op)
    dve_ops.CUSTOM_DVE_SPECS[name] = spec
    dve_ops._SUB_OPCODE_FOR_NAME[name] = row
    for ver in ("v3", "v4"):
        compiled = DveOpSpec(
            name=name, opcode=row, uops=lower(spec, ver=ver),
            rd1_en=_has_src1(spec),
        )
        op.uops_sha[ver] = compiled.sha(ver)
    return op


ABSDIFF2_MIN = _register_absdiff2_min_op()

F32 = mybir.dt.float32
EPS = 1e-8

B, H, W = 4, 64, 64
N = H * W              # 4096 points per batch
HALF_ROWS = 32         # image rows per core
NI = HALF_ROWS * W     # 2048 gradient points per core
NTILES = NI // 128     # 16 i-tiles per core
NBLK = N // 128        # 32 j-tiles in pass B

def build_nc():
    nc = bacc.Bacc("TRN2", target_bir_lowering=False, debug=False)

    x_dram = nc.dram_tensor("xsh", [W, 3 * (HALF_ROWS + 2)], F32, kind="ExternalInput")
    b_dram = nc.dram_tensor("bvec", [N], F32, kind="ExternalInput")
    g_scr = nc.dram_tensor("gscratch", [NI], F32)
    rowsum_dram = nc.dram_tensor("rowsum", [128, 1], F32, kind="ExternalOutput")
    colmin_dram = nc.dram_tensor("colmin", [128, NBLK], F32, kind="ExternalOutput")

    with tile.TileContext(nc) as tc:
        with (
            tc.tile_pool(name="consts", bufs=1) as consts,
            tc.tile_pool(name="sobel", bufs=1) as sobel,
            tc.tile_pool(name="bigbuf", bufs=1) as bigbuf,
            tc.tile_pool(name="psum_big", bufs=1, space="PSUM") as psum_big,
            tc.tile_pool(name="outs", bufs=1) as outs,
        ):
            # ---- Sobel, transposed layout (image cols on partitions). The
            # host supplies three column-shifted copies of the padded slab
            # (xm1 | x0 | xp1) so no cross-partition shifts are needed;
            # vertical taps are free-axis shifts.
            RP = HALF_ROWS + 2
            xsh = sobel.tile([W, 3 * RP], F32)
            nc.sync.dma_start(out=xsh[:], in_=x_dram.ap())
            b_row = bigbuf.tile([1, N], F32)
            nc.sync.dma_start(out=b_row[:], in_=b_dram.ap().unsqueeze(0))
            ones = consts.tile([1, 128], F32)
            nc.vector.memset(ones[:], 1.0)

            # boundary broadcast, split across the two DVE streams:
            # lower half = ones ⊗ b[0:2048] via rank-1 PE matmul, resident in
            # PSUM; upper half = 8 stride-0 DMAs straight from DRAM to SBUF
            # (they start at t~0). The two-stream custom op reads one half
            # per port (src0/src1 cannot both be PSUM).
            ps_big = psum_big.tile([128, N // 2], F32)
            for u in range(4):
                nc.tensor.matmul(
                    ps_big[:, u * 512:(u + 1) * 512], ones[:],
                    b_row[:, u * 512:(u + 1) * 512], start=True, stop=True,
                )
            b_hi = bigbuf.tile([128, N // 2], F32)
            for q in range(8):
                nc.sync.dma_start(
                    out=b_hi[q * 16:(q + 1) * 16, :],
                    in_=b_dram.ap()[N // 2:N].partition_broadcast(16),
                )
            xm1, x0, xp1 = xsh[:, 0:RP], xsh[:, RP:2 * RP], xsh[:, 2 * RP:3 * RP]

            hd = sobel.tile([W, RP], F32)              # x[c-1] - x[c+1]
            nc.vector.tensor_tensor(hd[:], xm1, xp1, op=mybir.AluOpType.subtract)
            t1 = sobel.tile([W, RP], F32)
            nc.vector.tensor_add(t1[:], xm1, x0)
            t2 = sobel.tile([W, RP], F32)
            nc.vector.tensor_add(t2[:], x0, xp1)
            hs = sobel.tile([W, RP], F32)              # x[c-1] + 2x[c] + x[c+1]
            nc.vector.tensor_add(hs[:], t1[:], t2[:])

            # gx = vertical [1,2,1] on hd;  gy = vertical [1,0,-1] on hs
            pg = sobel.tile([W, HALF_ROWS + 1], F32)
            nc.vector.tensor_add(pg[:], hd[:, 0:HALF_ROWS + 1], hd[:, 1:HALF_ROWS + 2])
            gx = sobel.tile([W, HALF_ROWS], F32)
            nc.vector.tensor_add(gx[:], pg[:, 0:HALF_ROWS], pg[:, 1:HALF_ROWS + 1])
            gy = sobel.tile([W, HALF_ROWS], F32)
            nc.vector.tensor_tensor(
                gy[:], hs[:, 0:HALF_ROWS], hs[:, 2:HALF_ROWS + 2],
                op=mybir.AluOpType.subtract,
            )

            gx2 = sobel.tile([W, HALF_ROWS], F32)
            nc.vector.tensor_tensor(gx2[:], gx[:], gx[:], op=mybir.AluOpType.mult)
            gy2 = sobel.tile([W, HALF_ROWS], F32)
            nc.vector.tensor_tensor(gy2[:], gy[:], gy[:], op=mybir.AluOpType.mult)
            ssum = sobel.tile([W, HALF_ROWS], F32)
            nc.vector.scalar_tensor_tensor(
                ssum[:], gx2[:], EPS, gy2[:],
                op0=mybir.AluOpType.add, op1=mybir.AluOpType.add,
            )
            gT = sobel.tile([W, HALF_ROWS], F32)
            nc.scalar.activation(
                gT[:], ssum[:], mybir.ActivationFunctionType.Sqrt, bias=0.0
            )

            # g_s (128, 16): i-tile t = image rows {t, t+16};
            # partition p<64 -> (row t, col p); p>=64 -> (row t+16, col p-64)
            g_s = consts.tile([128, NTILES], F32)
            nc.vector.tensor_copy(g_s[0:64, :], gT[:, 0:NTILES])
            nc.vector.tensor_copy(g_s[64:128, :], gT[:, NTILES:2 * NTILES])

            # g broadcast for pass B: flatten gT to one partition (DMA),
            # then rank-1 PE broadcast to (128, 2048), like b_bcast.
            nc.sync.dma_start(out=g_scr.ap(), in_=gT[:])
            g_bcast = bigbuf.tile([128, NI], F32)
            for q in range(8):
                nc.sync.dma_start(
                    out=g_bcast[q * 16:(q + 1) * 16, :],
                    in_=g_scr.ap().partition_broadcast(16),
                )

            # b per-partition scalars for pass B: b_s[p, u] = b[p*32 + u]
            b_s = consts.tile([128, NBLK], F32)
            nc.sync.dma_start(
                out=b_s[:], in_=b_dram.ap().rearrange("(p u) -> p u", p=128)
            )


            # ---- the two min passes, all on the two-stream fused DVE op:
            # one instruction per tile computes min(|in0-s0|,|in1-s0|)
            # elementwise (one half of the reduced axis per read port, so 2
            # elements/cycle) and min-reduces it into accum_out. The first
            # two pass-A tiles are further split into quarter chunks chained
            # through s1 so DVE starts as soon as the first broadcast
            # matmuls/DMAs land.
            BIG = 3.0e38
            junk = bigbuf.tile([128, N // 2], F32)

            rowmin_s = outs.tile([128, NTILES], F32)
            colmin_s = outs.tile([128, NBLK], F32)

            for t in range(NTILES):
                if t < 2:
                    nc.vector._custom_dve(
                        ABSDIFF2_MIN, out=junk[:, 0:1024],
                        accum_out=rowmin_s[:, t:t + 1],
                        in0=ps_big[:, 0:1024], in1=b_hi[:, 0:1024],
                        s0=g_s[:, t:t + 1], s1=BIG,
                    )
                    nc.vector._custom_dve(
                        ABSDIFF2_MIN, out=junk[:, 1024:2048],
                        accum_out=rowmin_s[:, t:t + 1],
                        in0=ps_big[:, 1024:2048], in1=b_hi[:, 1024:2048],
                        s0=g_s[:, t:t + 1], s1=rowmin_s[:, t:t + 1],
                    )
                else:
                    nc.vector._custom_dve(
                        ABSDIFF2_MIN, out=junk[:],
                        accum_out=rowmin_s[:, t:t + 1],
                        in0=ps_big[:], in1=b_hi[:],
                        s0=g_s[:, t:t + 1], s1=BIG,
                    )

            for u in range(NBLK):
                nc.vector._custom_dve(
                    ABSDIFF2_MIN, out=junk[:, 0:NI // 2],
                    accum_out=colmin_s[:, u:u + 1],
                    in0=g_bcast[:, 0:NI // 2], in1=g_bcast[:, NI // 2:NI],
                    s0=b_s[:, u:u + 1], s1=BIG,
                )

            # ---- outputs
            rsum = outs.tile([128, 1], F32)
            nc.vector.tensor_reduce(
                rsum[:], rowmin_s[:], axis=mybir.AxisListType.X,
                op=mybir.AluOpType.add,
            )
            nc.sync.dma_start(out=rowsum_dram.ap(), in_=rsum[:])
            nc.sync.dma_start(out=colmin_dram.ap(), in_=colmin_s[:])

    nc.compile()
    return nc


_NC = None


def _get_nc():
    global _NC
    if _NC is None:
        _NC = build_nc()
    return _NC


def make_in_maps(depth_pred: np.ndarray, boundary_gt: np.ndarray):
    depth = np.asarray(depth_pred, np.float32).reshape(B, H, W)
    bnd = np.asarray(boundary_gt, np.float32).reshape(B, N)
    in_maps = []
    for k in range(8):
        bi, h = k // 2, k % 2
        r0 = h * HALF_ROWS
        slab = np.zeros((HALF_ROWS + 2, W), np.float32)  # rows r0-1 .. r0+32
        lo, hi = max(r0 - 1, 0), min(r0 + HALF_ROWS + 1, H)
        slab[lo - (r0 - 1):hi - (r0 - 1), :] = depth[bi, lo:hi, :]
        # three column-shifted copies: xsh[c] = [slab[:,c-1], slab[:,c], slab[:,c+1]]
        xsh = np.zeros((W, 3, HALF_ROWS + 2), np.float32)
        xsh[1:, 0, :] = slab[:, 0:W - 1].T
        xsh[:, 1, :] = slab.T
        xsh[0:W - 1, 2, :] = slab[:, 1:W].T
        in_maps.append({
            "xsh": np.ascontiguousarray(xsh.reshape(W, 3 * (HALF_ROWS + 2))),
            "bvec": np.ascontiguousarray(bnd[bi]),
        })
    return in_maps


def combine(results):
    dist1 = 0.0
    dist2 = 0.0
    for bi in range(B):
        dist1 += float(results[2 * bi]["rowsum"].sum(dtype=np.float64))
        dist1 += float(results[2 * bi + 1]["rowsum"].sum(dtype=np.float64))
        cm = np.minimum(results[2 * bi]["colmin"], results[2 * bi + 1]["colmin"])
        dist2 += float(cm.sum(dtype=np.float64))
    return np.float32(dist1 / (B * N) + dist2 / (B * N))


def kernel(depth_pred: np.ndarray, boundary_gt: np.ndarray) -> np.ndarray:
    nc = _get_nc()
    in_maps = make_in_maps(depth_pred, boundary_gt)
    try:
        res = run_bass_kernel_spmd(nc, in_maps, core_ids=list(range(8)))
    except Exception:
        # transient NRT device wedge: reset the PJRT backend (equivalent to
        # a fresh process touching jax.devices()), back off, retry once
        import time
        try:
            import jax
            import jax._src.xla_bridge as _xb
            _xb._clear_backends() if hasattr(_xb, "_clear_backends") else None
            jax.clear_caches()
            jax.devices()
        except Exception:
            pass
        time.sleep(20)
        res = run_bass_kernel_spmd(nc, in_maps, core_ids=list(range(8)))
    return combine(res.results)



# revision 6
# speedup vs baseline: 1.7115x; 1.7115x over previous
"""Chamfer-like distance loss on Trainium2 (Bass/Tile), 8-core SPMD — v2.

Problem: depth_pred (4,1,64,64), boundary_gt (4,1,64,64).
  g = sqrt(sobel_x(depth)^2 + sobel_y(depth)^2 + 1e-8)  flattened to (B, N=4096)
  b = boundary flattened (B, 4096)
  out = mean_i min_j |g_i - b_j| + mean_j min_i |g_i - b_j|

v2 algorithm — 1-D grid quantization instead of the O(N^2) brute force.
For a query set Q against a data set D on the real line, place grid cells
with centers c_m (spacing h) covering D's range and compute
    e_m = min_{d in D} |d - c_m|         (one abs-min reduce per cell tile)
    d(q) ~= min_m (|q - c_m| + e_m)      (min over M cells, not |D| points)
The triangle inequality gives d_true <= d_hat <= d_true + h, so d_hat - h/2
has error <= h/2 per point — far inside the 2e-2 relative tolerance on the
final scalar (abs budget ~0.066; worst case h_b/2 + h_g/2 = 0.023;
measured end-to-end error ~2.6e-3 on the reference inputs).

Per core k (batch k//2, half k%2): sobel over the FULL image in transposed
layout (host ships column-shifted slabs; odd-half cores get the 180deg
ROTATED image — sobel magnitude is rot180-invariant, and the rotated
image's first 32 rows are the original's last 32, so one fixed program
serves both halves). dist1: e_b over all 4096 b's on a 64-cell grid [0,1]
(centers duplicated across the two partition halves, each handling half
the data; partial mins recombined after a PE transpose), then 16 QDIST
tiles for the core's 2048 g queries. dist2: e_g over all 4096 g's on a
256-cell grid [0,8] (2 tiles), then 16 QDIST tiles for its 2048 b queries.
Queries are pre-scaled to grid units (q' = q/h - 1/2) so the cell center
inside the custom op is just the element index:
    QDIST_MIN_ANT: out = |Idx - s0| + in0, accum = min(s1, min_k out)
Broadcast payloads (b replicated from host, g bounced via DRAM) are fp16
to halve DMA bytes; all arithmetic is fp32. Host combine: two sums plus
the -h/2 bias corrections.
"""
import os
import sys

import numpy as np

for _p in ("/opt/trn_rl_repo", os.path.expanduser("~/.axon_site/_ro/trn_rl_repo")):
    if os.path.isdir(_p) and _p not in sys.path:
        sys.path.insert(0, _p)

import concourse.bass as bass
import concourse.bacc as bacc
import concourse.tile as tile
from concourse import mybir
from concourse.bass_utils import run_bass_kernel_spmd
from concourse import dve_ops
from concourse.dve_spec import (
    Spec, Src0, Src1, C0, C1, Idx, maxx, minn, lower, _has_src1,
)
from concourse.dve_uop import DveOpSpec


def _register_op(name, spec):
    for o in dve_ops.OPS:
        if o.name == name:
            return o
    op = dve_ops.DveOp(name, spec, subdim=False, uops_sha={})
    row = dve_ops._CUSTOM_DVE_ROW_BASE + len(dve_ops.OPS)
    assert row < 0x20
    dve_ops.OPS.append(op)
    dve_ops.CUSTOM_DVE_SPECS[name] = spec
    dve_ops._SUB_OPCODE_FOR_NAME[name] = row
    for ver in ("v3", "v4"):
        compiled = DveOpSpec(
            name=name, opcode=row, uops=lower(spec, ver=ver),
            rd1_en=_has_src1(spec),
        )
        op.uops_sha[ver] = compiled.sha(ver)
    return op


def _ref_abs2(in0, in1, s0, s1, imm2):
    b = np.minimum(
        np.abs(in0.astype(np.float32) - s0),
        np.abs(in1.astype(np.float32) - s0),
    ).astype(np.float32)
    acc = np.minimum(
        np.float32(s1) if np.isscalar(s1) else s1.astype(np.float32),
        b.reshape(b.shape[0], -1).min(axis=-1, keepdims=True),
    )
    return b, acc


# two-stream fused abs-diff min reduce: out = min(|in0-s0|, |in1-s0|),
# accum = min(s1, min_k out). Both read ports stream data.
ABS2_MIN = _register_op(
    "ABS2_MIN_RED_ANT",
    Spec(
        body=minn(maxx(Src0 - C0, C0 - Src0), maxx(Src1 - C0, C0 - Src1)),
        accum=minn,
        accum_init=C1,
        reference=_ref_abs2,
    ),
)


def _ref_qdist(in0, in1, s0, s1, imm2):
    P, NN = in0.shape[0], int(np.prod(in0.shape[1:]))
    e = in0.astype(np.float32).reshape(P, NN)
    idx = np.arange(NN, dtype=np.float32)[None, :]
    body = (np.abs(idx - s0) + e).astype(np.float32)
    acc = np.minimum(
        np.float32(s1) if np.isscalar(s1) else s1.astype(np.float32),
        body.min(axis=-1, keepdims=True),
    )
    return body, acc


# grid nearest-cell query: out = |Idx - s0| + in0, accum = min(s1, min out).
# s0 is the query in grid units; in0 carries e_m (cell residuals, grid units).
QDIST_MIN = _register_op(
    "QDIST_MIN_ANT",
    Spec(
        body=maxx(Idx - C0, C0 - Idx) + Src0,
        accum=minn,
        accum_init=C1,
        reference=_ref_qdist,
    ),
)

F32 = mybir.dt.float32
F16 = mybir.dt.float16
EPS = 1e-8
BIG = 3.0e38

B, H, W = 4, 64, 64
N = H * W              # 4096 points per batch
RP = H + 2             # padded rows in the sobel slab
NT = 16                # 16 query tiles of 128 per side

MB = 64                # b-grid cells on [0, 1]
HB = 1.0 / MB
MG = 256               # g-grid cells on [0, 8]
HG = 8.0 / MG


def build_nc():
    nc = bacc.Bacc("TRN2", target_bir_lowering=False, debug=False)

    x_dram = nc.dram_tensor("xsh", [W, 3 * RP], F32, kind="ExternalInput")
    bb_dram = nc.dram_tensor("bb", [128, N // 2], F16, kind="ExternalInput")
    bvec_dram = nc.dram_tensor("bvec", [N // 2], F32, kind="ExternalInput")
    cb_dram = nc.dram_tensor("cb", [128, 1], F32, kind="ExternalInput")
    cg_dram = nc.dram_tensor("cg", [128, 2], F32, kind="ExternalInput")
    ones_dram = nc.dram_tensor("ones", [1, 128], F32, kind="ExternalInput")
    ident_dram = nc.dram_tensor("ident", [128, 128], F32, kind="ExternalInput")
    g_scr = nc.dram_tensor("gscratch", [N], F16)
    d1_dram = nc.dram_tensor("d1sum", [128, 1], F32, kind="ExternalOutput")
    d2_dram = nc.dram_tensor("d2sum", [128, 1], F32, kind="ExternalOutput")

    with tile.TileContext(nc) as tc:
        with (
            tc.tile_pool(name="consts", bufs=1) as consts,
            tc.tile_pool(name="sobel", bufs=1) as sobel,
            tc.tile_pool(name="bigbuf", bufs=1) as bigbuf,
            tc.tile_pool(name="psum", bufs=1, space="PSUM") as psum,
            tc.tile_pool(name="outs", bufs=1) as outs,
        ):
            # ---- input DMAs (all issued up front)
            xsh = sobel.tile([W, 3 * RP], F32)
            nc.sync.dma_start(out=xsh[:], in_=x_dram.ap())
            # bb rows 0-63: b[0:2048] replicated; rows 64-127: b[2048:4096]
            bb = bigbuf.tile([128, N // 2], F16)
            for q in range(8):
                nc.sync.dma_start(
                    out=bb[q * 16:(q + 1) * 16, :],
                    in_=bb_dram.ap()[q * 16:(q + 1) * 16, :],
                )
            cb = consts.tile([128, 1], F32)
            nc.sync.dma_start(out=cb[:], in_=cb_dram.ap())
            cg = consts.tile([128, 2], F32)
            nc.sync.dma_start(out=cg[:], in_=cg_dram.ap())
            ones = consts.tile([1, 128], F32)
            nc.sync.dma_start(out=ones[:], in_=ones_dram.ap())
            ident = consts.tile([128, 128], F32)
            nc.sync.dma_start(out=ident[:], in_=ident_dram.ap())
            # the core's 2048 b queries (its half of the batch's b vector)
            b_s = consts.tile([128, NT], F32)
            nc.sync.dma_start(
                out=b_s[:], in_=bvec_dram.ap().rearrange("(p u) -> p u", p=128)
            )

            # ---- Sobel over the full image, transposed layout (cols on
            # partitions, rows on the free axis; vertical taps = free shifts).
            xm1, x0, xp1 = xsh[:, 0:RP], xsh[:, RP:2 * RP], xsh[:, 2 * RP:3 * RP]
            hd = sobel.tile([W, RP], F32)
            nc.vector.tensor_tensor(hd[:], xm1, xp1, op=mybir.AluOpType.subtract)
            t1 = sobel.tile([W, RP], F32)
            nc.vector.tensor_add(t1[:], xm1, x0)
            t2 = sobel.tile([W, RP], F32)
            nc.vector.tensor_add(t2[:], x0, xp1)
            hs = sobel.tile([W, RP], F32)
            nc.vector.tensor_add(hs[:], t1[:], t2[:])

            pg = sobel.tile([W, H + 1], F32)
            nc.vector.tensor_add(pg[:], hd[:, 0:H + 1], hd[:, 1:H + 2])
            gx = sobel.tile([W, H], F32)
            nc.vector.tensor_add(gx[:], pg[:, 0:H], pg[:, 1:H + 1])
            gy = sobel.tile([W, H], F32)
            nc.vector.tensor_tensor(
                gy[:], hs[:, 0:H], hs[:, 2:H + 2], op=mybir.AluOpType.subtract
            )
            gx2 = sobel.tile([W, H], F32)
            nc.vector.tensor_tensor(gx2[:], gx[:], gx[:], op=mybir.AluOpType.mult)
            gy2 = sobel.tile([W, H], F32)
            nc.vector.tensor_tensor(gy2[:], gy[:], gy[:], op=mybir.AluOpType.mult)
            ssum = sobel.tile([W, H], F32)
            nc.vector.scalar_tensor_tensor(
                ssum[:], gx2[:], EPS, gy2[:],
                op0=mybir.AluOpType.add, op1=mybir.AluOpType.add,
            )
            gT = sobel.tile([W, H], F32)  # gT[c, r] = g at image (row r, col c)
            nc.scalar.activation(
                gT[:], ssum[:], mybir.ActivationFunctionType.Sqrt, bias=0.0
            )

            # fp16 copy of g for the DRAM bounce + broadcast (dist2 data)
            gT16 = sobel.tile([W, H], F16)
            nc.vector.tensor_copy(gT16[:], gT[:])
            nc.sync.dma_start(out=g_scr.ap(), in_=gT16[:])
            g_bcast = bigbuf.tile([128, N], F16)
            for q in range(8):
                nc.sync.dma_start(
                    out=g_bcast[q * 16:(q + 1) * 16, :],
                    in_=g_scr.ap().partition_broadcast(16),
                )

            # d1 query scalars: the core's half = image rows 0..31 of the
            # (possibly rot180'd) shipped image = gT free columns 0..31.
            g_s = consts.tile([128, NT], F32)
            nc.vector.tensor_copy(g_s[0:64, :], gT[:, 0:NT])
            nc.vector.tensor_copy(g_s[64:128, :], gT[:, NT:2 * NT])
            g_q = consts.tile([128, NT], F32)
            nc.vector.tensor_scalar(
                g_q[:], g_s[:], 1.0 / HB, -0.5,
                op0=mybir.AluOpType.mult, op1=mybir.AluOpType.add,
            )
            b_q = consts.tile([128, NT], F32)
            nc.vector.tensor_scalar(
                b_q[:], b_s[:], 1.0 / HG, -0.5,
                op0=mybir.AluOpType.mult, op1=mybir.AluOpType.add,
            )

            junk = bigbuf.tile([128, N // 2], F32)
            junkq = bigbuf.tile([128, MG], F32)

            # ---- dist1 cell pass: e_b over all 4096 b's. 64 centers live on
            # partitions 0-63 AND (duplicated) 64-127; the partition halves
            # read disjoint halves of b (bb row layout), ports split again.
            e_pair = outs.tile([128, 1], F32)
            nc.vector._custom_dve(
                ABS2_MIN, out=junk[:, 0:1024],
                accum_out=e_pair[:],
                in0=bb[:, 0:1024], in1=bb[:, 1024:2048],
                s0=cb[:], s1=BIG,
            )

            ps_erow = psum.tile([1, 128], F32)
            nc.tensor.matmul(ps_erow[:], e_pair[:], ident[:], start=True, stop=True)
            e_row_sb = consts.tile([1, 128], F32)
            nc.vector.tensor_scalar(
                e_row_sb[:], ps_erow[:], 1.0 / HB, None, op0=mybir.AluOpType.mult
            )
            e_brow = consts.tile([1, MB], F32)
            nc.vector.tensor_tensor(
                e_brow[:], e_row_sb[0:1, 0:64], e_row_sb[0:1, 64:128],
                op=mybir.AluOpType.min,
            )
            ps_ebb = psum.tile([128, MB], F32)
            nc.tensor.matmul(ps_ebb[:], ones[:], e_brow[:], start=True, stop=True)

            # ---- dist1 queries
            r1 = outs.tile([128, NT], F32)
            for t in range(NT):
                nc.vector._custom_dve(
                    QDIST_MIN, out=junkq[:, 0:MB],
                    accum_out=r1[:, t:t + 1],
                    in0=ps_ebb[:], s0=g_q[:, t:t + 1], s1=BIG,
                )

            # ---- dist2 cell pass: e_g over all 4096 g's (fp16 broadcast),
            # 2 tiles of 128 cells each reading the full free axis.
            e_g = outs.tile([128, 2], F32)
            for u in range(2):
                nc.vector._custom_dve(
                    ABS2_MIN, out=junk[:, 0:2048],
                    accum_out=e_g[:, u:u + 1],
                    in0=g_bcast[:, 0:2048], in1=g_bcast[:, 2048:4096],
                    s0=cg[:, u:u + 1], s1=BIG,
                )

            ps_egrow = psum.tile([1, MG], F32)
            nc.tensor.matmul(
                ps_egrow[0:1, 0:128], e_g[:, 0:1], ident[:], start=True, stop=True
            )
            nc.tensor.matmul(
                ps_egrow[0:1, 128:256], e_g[:, 1:2], ident[:], start=True, stop=True
            )
            e_grow = consts.tile([1, MG], F32)
            nc.vector.tensor_scalar(
                e_grow[:], ps_egrow[:], 1.0 / HG, None, op0=mybir.AluOpType.mult
            )
            ps_egb = psum.tile([128, MG], F32)
            nc.tensor.matmul(ps_egb[:], ones[:], e_grow[:], start=True, stop=True)

            # ---- dist2 queries
            r2 = outs.tile([128, NT], F32)
            for t in range(NT):
                nc.vector._custom_dve(
                    QDIST_MIN, out=junkq[:, 0:MG],
                    accum_out=r2[:, t:t + 1],
                    in0=ps_egb[:], s0=b_q[:, t:t + 1], s1=BIG,
                )

            # ---- finals: per-partition sums, scaled back to absolute units
            rs1 = outs.tile([128, 1], F32)
            nc.vector.tensor_reduce(
                rs1[:], r1[:], axis=mybir.AxisListType.X, op=mybir.AluOpType.add
            )
            d1o = outs.tile([128, 1], F32)
            nc.vector.tensor_scalar(
                d1o[:], rs1[:], HB, None, op0=mybir.AluOpType.mult
            )
            nc.sync.dma_start(out=d1_dram.ap(), in_=d1o[:])
            rs2 = outs.tile([128, 1], F32)
            nc.vector.tensor_reduce(
                rs2[:], r2[:], axis=mybir.AxisListType.X, op=mybir.AluOpType.add
            )
            d2o = outs.tile([128, 1], F32)
            nc.vector.tensor_scalar(
                d2o[:], rs2[:], HG, None, op0=mybir.AluOpType.mult
            )
            nc.sync.dma_start(out=d2_dram.ap(), in_=d2o[:])

    nc.compile()
    return nc


_NC = None


def _get_nc():
    global _NC
    if _NC is None:
        _NC = build_nc()
    return _NC


def make_in_maps(depth_pred: np.ndarray, boundary_gt: np.ndarray):
    depth = np.asarray(depth_pred, np.float32).reshape(B, H, W)
    bnd = np.asarray(boundary_gt, np.float32).reshape(B, N)

    cb = np.zeros((128, 1), np.float32)
    cb[0:64, 0] = (np.arange(64) + 0.5) * HB
    cb[64:128, 0] = (np.arange(64) + 0.5) * HB
    cg = np.zeros((128, 2), np.float32)
    cg[:, 0] = (np.arange(128) + 0.5) * HG
    cg[:, 1] = (np.arange(128, 256) + 0.5) * HG
    ones = np.ones((1, 128), np.float32)
    ident = np.eye(128, dtype=np.float32)

    in_maps = []
    for k in range(8):
        bi, h = k // 2, k % 2
        img = depth[bi] if h == 0 else depth[bi][::-1, ::-1]
        # padded slab rows -1..64 of the (possibly rotated) image
        slab = np.zeros((RP, W), np.float32)
        slab[1:RP - 1, :] = img
        # three column-shifted copies: xsh[c] = [slab[:,c-1], slab[:,c], slab[:,c+1]]
        xsh = np.zeros((W, 3, RP), np.float32)
        xsh[1:, 0, :] = slab[:, 0:W - 1].T
        xsh[:, 1, :] = slab.T
        xsh[0:W - 1, 2, :] = slab[:, 1:W].T

        bhalf = np.empty((128, N // 2), np.float16)
        bhalf[0:64, :] = bnd[bi, 0:2048].astype(np.float16)[None, :]
        bhalf[64:128, :] = bnd[bi, 2048:4096].astype(np.float16)[None, :]

        in_maps.append({
            "xsh": np.ascontiguousarray(xsh.reshape(W, 3 * RP)),
            "bb": bhalf,
            "bvec": np.ascontiguousarray(bnd[bi, h * 2048:(h + 1) * 2048]),
            "cb": cb,
            "cg": cg,
            "ones": ones,
            "ident": ident,
        })
    return in_maps


def combine(results):
    d1 = 0.0
    d2 = 0.0
    for k in range(8):
        d1 += float(results[k]["d1sum"].sum(dtype=np.float64))
        d2 += float(results[k]["d2sum"].sum(dtype=np.float64))
    dist1 = d1 / (B * N) - HB / 2
    dist2 = d2 / (B * N) - HG / 2
    return np.float32(dist1 + dist2)


def kernel(depth_pred: np.ndarray, boundary_gt: np.ndarray) -> np.ndarray:
    nc = _get_nc()
    in_maps = make_in_maps(depth_pred, boundary_gt)
    try:
        res = run_bass_kernel_spmd(nc, in_maps, core_ids=list(range(8)))
    except Exception:
        # transient NRT device wedge: reset the PJRT backend (equivalent to
        # a fresh process touching jax.devices()), back off, retry once
        import time
        try:
            import jax
            import jax._src.xla_bridge as _xb
            _xb._clear_backends() if hasattr(_xb, "_clear_backends") else None
            jax.clear_caches()
            jax.devices()
        except Exception:
            pass
        time.sleep(20)
        res = run_bass_kernel_spmd(nc, in_maps, core_ids=list(range(8)))
    return combine(res.results)
